# revision 1
# baseline (speedup 1.0000x reference)
"""E3GNN message-passing layer on 8 TRN2 NeuronCores (Bass/Tile).

Strategy (edge data parallelism by *source-node range*):
  - Sort edges by src; core i owns src in [1250*i, 1250*(i+1)).  Each core
    fully aggregates its own nodes' messages, so NO cross-core collective is
    needed: each core returns out[1250*i : 1250*(i+1)] and the host stacks.
  - Per-edge layer-1 input is decomposed:  e_in @ We1 =
        P[src] + Q[dst] + latproj[g] + dis @ Wd   (+ be1 folded into P)
    with P = nf @ We1[0:256] + be1 (local rows only), Q = nf @ We1[256:512]
    (all rows, computed on every core), latproj = lat_ips @ We1[512:521],
    Wd = We1[521:581].  P/Q/lat tables are built on device in bf16.
  - Q and latproj rows are fetched per edge with gpsimd.dma_gather
    (transpose=True) -> feature-major tiles that accumulate into PSUM via
    identity matmuls.  P[src] is expanded via host-provided one-hot
    (edges sorted => src spans one 128-node window per 128-edge subtile).
  - Layer 2 swaps matmul operand roles (lhsT = x1^T tile) so x2 lands
    edge-major for the scatter, with zero explicit transposes.
  - Scatter-mean: one-hot^T @ [x2 | 1] accumulated in PSUM per 128-node
    window (sorted edges => sequential windows), then the node MLP.
Compute dtype bf16 (fp32 PSUM accumulation); layer 2 and the scatter
matmul run in fp8-e4m3 DoubleRow (2x PE) - rel err stays ~2.4e-4, far
under the 2e-2 gate.  Sinusoid range reduction uses the 2^23 magic
constant (RNE round) since ScalarE Sin only covers [-pi, pi].
Note: gpsimd dma_gather with num_idxs=1024 crashes the NEFF on silicon
(NRT_EXEC_UNIT_UNRECOVERABLE); keep num_idxs <= 512 per call.
"""

import os
import sys

import numpy as np

sys.path.insert(0, "/opt/trn_rl_repo")

import ml_dtypes  # noqa: E402

import concourse.bass as bass  # noqa: E402
import concourse.mybir as mybir  # noqa: E402
import concourse.tile as tile  # noqa: E402
from concourse import bacc, library_config  # noqa: E402
from concourse.masks import make_identity  # noqa: E402

# ---- problem constants (hardcoded per contract) ----
N_NODES = 10000
N_EDGES = 200000
N_GRAPHS = 500
HID = 256
NUM_FREQS = 10
N_CORES = 8
NPC = N_NODES // N_CORES          # nodes per core = 1250
WSZ = 128                         # node-window size
NW = 10                           # windows per core (1280 slots, 30 unused)
NLOC = NW * WSZ                   # 1280 local node slots
TE = 512                          # edges per pipeline tile
NSPT = TE // 128                  # 4 subtiles per tile
G_PAD = 512                       # lattice table rows (500 real)
USE_FP8_L2 = os.environ.get("K_FP8_L2", "1") == "1"
USE_FP8_SCT = os.environ.get("K_FP8_SCT", "1") == "1"
USE_GATHER1 = os.environ.get("K_GATHER1", "0") == "1"

F32 = mybir.dt.float32
F8 = mybir.dt.float8e4
BF16 = mybir.dt.bfloat16
I16 = mybir.dt.int16
BF = ml_dtypes.bfloat16
F8NP = ml_dtypes.float8_e4m3fn
ALU = None  # set after import in functions


# --------------------------------------------------------------------------
# host-side sharding / index prep (pure indexing, no FP math on data)
# --------------------------------------------------------------------------

def _prep(inputs):
    nf = np.asarray(inputs["node_features"], np.float32)
    frac = np.asarray(inputs["frac_coords"], np.float32)
    lat = np.asarray(inputs["lattices"], np.float32)
    ei = np.asarray(inputs["edge_index"], np.int64)
    e2g = np.asarray(inputs["edge2graph"], np.int64)
    src, dst = ei[0], ei[1]

    core = src // NPC
    loc = src - core * NPC
    win = loc // WSZ
    order = np.lexsort((e2g, win, core))  # sort by (core, window, graph)
    src_s, dst_s, g_s = src[order], dst[order], e2g[order]
    core_s, loc_s, win_s = core[order], loc[order], win[order]

    # per (core, window) edge counts -> shared static subtile counts
    cnts = np.zeros((N_CORES, NW), np.int64)
    for c in range(N_CORES):
        cw = win_s[core_s == c]
        for w in range(NW):
            cnts[c, w] = int((cw == w).sum())
    sub_w = np.maximum(1, -(-cnts.max(axis=0) // 128))  # ceil, >=1
    n_sub = int(sub_w.sum())
    n_sub = -(-n_sub // NSPT) * NSPT                    # round to tile
    sub_w[NW - 1] += n_sub - int(sub_w.sum())
    e_pad = n_sub * 128
    n_tiles = n_sub // NSPT
    wstart = np.concatenate(([0], np.cumsum(sub_w)))    # window->subtile range

    per_core = []
    for c in range(N_CORES):
        m = core_s == c
        csrc_rel = (loc_s[m] - win_s[m] * WSZ)
        cdst, cg, cwin = dst_s[m], g_s[m], win_s[m]
        dsti = np.zeros(e_pad, np.int64)
        gi = np.zeros(e_pad, np.int64)
        srel = np.full(e_pad, -1, np.int64)
        fsrc = np.zeros((e_pad, 3), np.float32)
        fdst = np.zeros((e_pad, 3), np.float32)
        for w in range(NW):
            wm = cwin == w
            k = int(wm.sum())
            s0 = wstart[w] * 128
            assert k <= (wstart[w + 1] - wstart[w]) * 128
            sl = slice(s0, s0 + k)
            dsti[sl] = cdst[wm]
            gi[sl] = cg[wm]
            srel[sl] = csrc_rel[wm]
            esrc = c * NPC + w * WSZ + csrc_rel[wm]
            fsrc[sl] = frac[esrc]
            fdst[sl] = frac[cdst[wm]]

        onehot = srel.reshape(n_sub, 128)[:, :, None] == np.arange(128)[None, None, :]
        sct = np.ascontiguousarray(onehot.transpose(1, 0, 2)).astype(
            F8NP if USE_FP8_SCT else BF)
        sctT = np.ascontiguousarray(onehot.transpose(2, 0, 1)).astype(BF)

        def wrap_idx(idx, per_call):
            # dma_gather idx layout: element t of a call lives at
            # [t % 16, t // 16]; replicate the 16-row block to 128 partitions.
            b = idx.reshape(n_tiles, per_call // 16, 16).transpose(0, 2, 1)
            b = np.ascontiguousarray(b.transpose(1, 0, 2)).astype(np.int16)
            return np.tile(b, (8, 1, 1))  # [128, n_tiles, per_call//16]

        ew = lambda a: np.ascontiguousarray(
            a.reshape(n_sub, 128, -1).transpose(1, 0, 2)).astype(np.float32)

        qg = np.concatenate(
            [dsti.reshape(n_tiles, TE), (gi + 10240).reshape(n_tiles, TE)],
            axis=1).reshape(-1)  # per tile: [dst(512) | 10240+g(512)]
        per_core.append(dict(
            qg_idx=wrap_idx(qg, 2 * TE),
            sct=sct, sctT=sctT,
            frac_s=ew(fsrc), frac_d=ew(fdst),
        ))

    # ---- shared (replicated) tensors ----
    nf_pad = np.zeros((10240, HID), np.float32)
    nf_pad[:N_NODES] = nf
    nfT_w = np.ascontiguousarray(
        nf_pad.reshape(10240, 2, 128).transpose(2, 1, 0)).astype(np.float32)
    lat_flat = np.zeros((G_PAD, 9), np.float32)
    lat_flat[:N_GRAPHS] = lat.reshape(N_GRAPHS, 9)
    latw = np.ascontiguousarray(
        lat_flat.reshape(4, 128, 9).transpose(1, 0, 2)).astype(np.float32)

    We1 = np.asarray(inputs["We1"], np.float32)
    wshape = lambda w, k: np.ascontiguousarray(
        w.reshape(k, 128, HID).transpose(1, 0, 2)).astype(np.float32)
    shared = dict(
        nfT=nfT_w, latw=latw,
        W1a=wshape(We1[0:256], 2), W1b=wshape(We1[256:512], 2),
        Wlat=np.ascontiguousarray(We1[512:521]).astype(np.float32),
        Wd=np.ascontiguousarray(We1[521:581]).astype(np.float32),
        W2=wshape(np.asarray(inputs["We2"], np.float32), 2),
        Wn1=wshape(np.asarray(inputs["Wn1"], np.float32), 4),
        Wn2=wshape(np.asarray(inputs["Wn2"], np.float32), 2),
        be1=np.asarray(inputs["be1"], np.float32).reshape(1, HID),
        be2=np.asarray(inputs["be2"], np.float32).reshape(1, HID),
        bn1=np.ascontiguousarray(
            np.asarray(inputs["bn1"], np.float32).reshape(2, 128).T),
        bn2=np.asarray(inputs["bn2"], np.float32).reshape(1, HID),
        freqs=np.tile(np.arange(NUM_FREQS, dtype=np.float32)
                      .reshape(1, NUM_FREQS), (128, 1)),
    )

    in_maps = []
    for c in range(N_CORES):
        m = dict(shared)
        m.update(per_core[c])
        m["nfT_loc"] = np.ascontiguousarray(nfT_w[:, :, c * NPC:c * NPC + NLOC])
        nl = np.zeros((NLOC, HID), np.float32)
        nl[:NPC] = nf[c * NPC:(c + 1) * NPC]
        m["nf_loc"] = np.ascontiguousarray(
            nl.reshape(NW, 128, HID).transpose(1, 0, 2))
        in_maps.append(m)

    meta = dict(n_sub=n_sub, n_tiles=n_tiles, wstart=[int(x) for x in wstart])
    return in_maps, meta


# --------------------------------------------------------------------------
# device program
# --------------------------------------------------------------------------

def build_program(meta):
    n_sub, n_tiles = meta["n_sub"], meta["n_tiles"]
    wstart = meta["wstart"]
    sub2w = np.zeros(n_sub, np.int64)
    for w in range(NW):
        sub2w[wstart[w]:wstart[w + 1]] = w
    S_IDX = TE // 16
    Alu = mybir.AluOpType
    Act = mybir.ActivationFunctionType

    nc = bacc.Bacc("TRN2", target_bir_lowering=False, debug=False)

    def din(name, shape, dt=F32):
        return nc.declare_dram_parameter(name, list(shape), dt, isOutput=False)

    nfT = din("nfT", (128, 2, 10240))
    nfT_loc = din("nfT_loc", (128, 2, NLOC))
    nf_loc = din("nf_loc", (128, NW, HID))
    latw = din("latw", (128, 4, 9))
    W1a = din("W1a", (128, 2, HID)); W1b = din("W1b", (128, 2, HID))
    Wlat = din("Wlat", (9, HID)); Wd = din("Wd", (60, HID))
    W2 = din("W2", (128, 2, HID))
    Wn1 = din("Wn1", (128, 4, HID)); Wn2 = din("Wn2", (128, 2, HID))
    be1 = din("be1", (1, HID)); be2 = din("be2", (1, HID))
    bn1 = din("bn1", (128, 2)); bn2 = din("bn2", (1, HID))
    freqs = din("freqs", (128, NUM_FREQS))
    qg_idx = din("qg_idx", (128, n_tiles, 2 * S_IDX), I16)
    sct_d = din("sct", (128, n_sub, 128), F8 if USE_FP8_SCT else BF16)
    sctT_d = din("sctT", (128, n_sub, 128), BF16)
    frac_s = din("frac_s", (128, n_sub, 3)); frac_d = din("frac_d", (128, n_sub, 3))
    out_d = nc.declare_dram_parameter("out", [128, NW, HID], F32, isOutput=True)

    q_dram = nc.dram_tensor("q_dram", [10240 + G_PAD, HID], BF16)

    with tile.TileContext(nc) as tc:
        nc.gpsimd.load_library(library_config.mlp)

        with (
            tc.tile_pool(name="persist", bufs=1) as pp,
            tc.tile_pool(name="stage", bufs=2) as sp,
        ):
            # ---------------- phase A: constants into SBUF ----------------
            ident = pp.tile([128, 128], BF16)
            make_identity(nc, ident[:])
            ones_row = pp.tile([1, 128], BF16)
            nc.gpsimd.memset(ones_row[:], 1.0)
            ones4 = pp.tile([128, NSPT], F8 if USE_FP8_SCT else BF16)
            nc.gpsimd.memset(ones4[:], 1.0)

            def load_bf(dram, shape, name, dt=BF16):
                t = pp.tile(list(shape), dt, name=name, tag=name)
                nc.gpsimd.dma_start(out=t[:], in_=dram[:])  # casts f32->dt
                return t

            W1a_s = load_bf(W1a, (128, 2, HID), "W1a_s")
            W1b_s = load_bf(W1b, (128, 2, HID), "W1b_s")
            Wlat_s = load_bf(Wlat, (9, HID), "Wlat_s")
            Wd_s = load_bf(Wd, (60, HID), "Wd_s")
            W2_s = load_bf(W2, (128, 2, HID), "W2_s", dt=F8 if USE_FP8_L2 else BF16)
            Wn1_s = load_bf(Wn1, (128, 4, HID), "Wn1_s")
            Wn2_s = load_bf(Wn2, (128, 2, HID), "Wn2_s")
            be1_s = load_bf(be1, (1, HID), "be1_s")
            be2_s = load_bf(be2, (1, HID), "be2_s")
            bn2_s = load_bf(bn2, (1, HID), "bn2_s")
            bn1_s = pp.tile([128, 2], F32)
            nc.sync.dma_start(out=bn1_s[:], in_=bn1[:])
            freqs_s = pp.tile([128, NUM_FREQS], F32)
            nc.sync.dma_start(out=freqs_s[:], in_=freqs[:])
            nfT_loc_s = load_bf(nfT_loc, (128, 2, NLOC), "nfT_loc_s")
            nf_loc_s = pp.tile([128, NW, HID], F32)
            nc.sync.dma_start(out=nf_loc_s[:], in_=nf_loc[:])

            # ---------------- phase B: node/graph tables ----------------
            P_sb = pp.tile([128, NW, HID], BF16)
            with tc.tile_pool(name="psB", bufs=2, space="PSUM") as psB:
                for w in range(NW):
                    ps = psB.tile([128, HID], F32, tag="tbl", bufs=4)
                    nc.tensor.matmul(out=ps[:], lhsT=ones_row[:],
                                     rhs=be1_s[:], start=True, stop=False)
                    for kc in range(2):
                        nc.tensor.matmul(
                            out=ps[:],
                            lhsT=nfT_loc_s[:, kc, w * 128:(w + 1) * 128],
                            rhs=W1a_s[:, kc, :], start=False, stop=(kc == 1))
                    nc.vector.tensor_copy(out=P_sb[:, w, :], in_=ps[:])

                with tc.tile_pool(name="nfull", bufs=1) as nfp:
                    NCH = 8
                    nfT_ch = []
                    for ci in range(NCH):
                        nft = nfp.tile([128, 2, 10240 // NCH], BF16,
                                       name=f"nfT{ci}", tag=f"nfT{ci}")
                        nc.gpsimd.dma_start(
                            out=nft[:],
                            in_=nfT[:, :, ci * 1280:(ci + 1) * 1280])
                        nfT_ch.append(nft)
                    for w in range(80):
                        ci, wl = w // 10, w % 10
                        ps = psB.tile([128, HID], F32, tag="tbl", bufs=4)
                        for kc in range(2):
                            nc.tensor.matmul(
                                out=ps[:],
                                lhsT=nfT_ch[ci][:, kc, wl * 128:(wl + 1) * 128],
                                rhs=W1b_s[:, kc, :], start=(kc == 0), stop=(kc == 1))
                        qs = sp.tile([128, HID], BF16, tag="qtile", bufs=8)
                        if w % 4 == 3:
                            nc.scalar.copy(out=qs[:], in_=ps[:])
                        else:
                            nc.vector.tensor_copy(out=qs[:], in_=ps[:])
                        eng = nc.sync if w % 2 == 0 else nc.scalar
                        eng.dma_start(out=q_dram[w * 128:(w + 1) * 128, :],
                                      in_=qs[:])

                # lattice inner products + projection table
                latc = pp.tile([128, 4, 9], F32)
                nc.sync.dma_start(out=latc[:], in_=latw[:])
                ipT_ps = psB.tile([9, 512], BF16, tag="ipT")
                for c4 in range(4):
                    a1 = sp.tile([128, 3, 3, 3], F32, tag="lat1")
                    a2 = sp.tile([128, 3, 3, 3], F32, tag="lat2")
                    lv = latc[:, c4, :].rearrange("p (i j) -> p i j", i=3)
                    nc.vector.tensor_copy(
                        out=a1[:], in_=lv.unsqueeze(2).to_broadcast([128, 3, 3, 3]))
                    nc.vector.tensor_copy(
                        out=a2[:], in_=lv.unsqueeze(1).to_broadcast([128, 3, 3, 3]))
                    nc.vector.tensor_tensor(out=a1[:], in0=a1[:], in1=a2[:],
                                            op=Alu.mult)
                    ip = sp.tile([128, 9], F32, tag="lat3")
                    pr = a1[:].rearrange("p i k j -> p (i k) j")
                    nc.vector.tensor_tensor(out=ip[:], in0=pr[:, :, 0],
                                            in1=pr[:, :, 1], op=Alu.add)
                    nc.vector.tensor_tensor(out=ip[:], in0=ip[:],
                                            in1=pr[:, :, 2], op=Alu.add)
                    ipb = sp.tile([128, 9], BF16, tag="lat4")
                    nc.vector.tensor_copy(out=ipb[:], in_=ip[:])
                    nc.tensor.transpose(out=ipT_ps[:, c4 * 128:(c4 + 1) * 128],
                                        in_=ipb[:], identity=ident[:])
                ipT_s = pp.tile([9, 512], BF16)
                nc.vector.tensor_copy(out=ipT_s[:], in_=ipT_ps[:])
                for c4 in range(4):
                    ps = psB.tile([128, HID], F32, tag="tbl", bufs=4)
                    nc.tensor.matmul(out=ps[:],
                                     lhsT=ipT_s[:, c4 * 128:(c4 + 1) * 128],
                                     rhs=Wlat_s[:], start=True, stop=True)
                    ls = sp.tile([128, HID], BF16, tag="qtile", bufs=8)
                    nc.vector.tensor_copy(out=ls[:], in_=ps[:])
                    nc.sync.dma_start(
                        out=q_dram[10240 + c4 * 128:10240 + (c4 + 1) * 128, :],
                        in_=ls[:])

            # ---------------- phase B2 prep: frac diff (whole) ----------------
            disT_tiles = {}
            diff = pp.tile([128, n_sub, 3], F32)
            with tc.tile_pool(name="fracp", bufs=1) as fp:
                fs = fp.tile([128, n_sub, 3], F32)
                fd = fp.tile([128, n_sub, 3], F32)
                nc.sync.dma_start(out=fs[:], in_=frac_s[:])
                nc.sync.dma_start(out=fd[:], in_=frac_d[:])
                neg = fp.tile([128, n_sub, 3], F32)
                nc.vector.tensor_tensor(out=diff[:], in0=fd[:], in1=fs[:],
                                        op=Alu.subtract)
                # floor-mod into [0,1): x += (x < 0)
                nc.vector.tensor_scalar(out=neg[:], in0=diff[:], scalar1=0.0,
                                        scalar2=None, op0=Alu.is_lt)
                nc.vector.tensor_tensor(out=diff[:], in0=diff[:], in1=neg[:],
                                        op=Alu.add)

            # ---------------- phase C: edge pipeline (B2 inlined) -------------
            qgi_s = pp.tile([128, n_tiles, 2 * S_IDX], I16)
            nc.sync.dma_start(out=qgi_s[:], in_=qg_idx[:])

            agg_bf = pp.tile([128, NW, HID], BF16)
            aggT = pp.tile([128, 2, NLOC], BF16)
            inv_cnt = pp.tile([128, NW], F32)

            CHK = 16  # subtiles per dis-chunk = 4 edge tiles
            MAGIC = 12582912.0  # 1.5 * 2**23

            def emit_dis_chunk(k):
                # sinusoid embeddings for subtiles [k*CHK, k*CHK+hsz),
                # written straight into disT_tiles (feature-major).
                g0 = k * CHK
                hsz = min(CHK, n_sub - g0)
                sl = slice(g0, g0 + hsz)
                emb = sp.tile([128, CHK, 3, NUM_FREQS], F32, tag="emb")
                nc.vector.tensor_tensor(
                    out=emb[:, :hsz],
                    in0=diff[:, sl, :].unsqueeze(3).to_broadcast(
                        [128, hsz, 3, NUM_FREQS]),
                    in1=freqs_s[:].unsqueeze(1).unsqueeze(1).to_broadcast(
                        [128, hsz, 3, NUM_FREQS]),
                    op=Alu.mult)
                # range-reduce via the 2^23 magic constant (RNE round):
                # r = y - round(y) in [-.5,.5]; sin(2pi*y) = Sin(2pi*r);
                # cos(2pi*y) = sin(2pi*(y+0.25)) the same way.
                ev = emb[:, :hsz].rearrange("p s c k -> p s (c k)")
                rm = sp.tile([128, CHK, 60], F32, tag="embm")
                t1 = sp.tile([128, CHK, 30], F32, tag="embt")
                t1v = t1[:, :hsz]
                nc.vector.tensor_scalar(out=t1v, in0=ev, scalar1=MAGIC,
                                        scalar2=None, op0=Alu.add)
                nc.vector.tensor_scalar(out=t1v, in0=t1v, scalar1=MAGIC,
                                        scalar2=None, op0=Alu.subtract)
                nc.vector.tensor_tensor(out=rm[:, :hsz, 0:30], in0=ev,
                                        in1=t1v, op=Alu.subtract)
                nc.vector.tensor_scalar(out=t1v, in0=ev, scalar1=0.25,
                                        scalar2=MAGIC, op0=Alu.add,
                                        op1=Alu.add)
                nc.vector.tensor_scalar(out=t1v, in0=t1v, scalar1=MAGIC,
                                        scalar2=0.25, op0=Alu.subtract,
                                        op1=Alu.subtract)
                nc.vector.tensor_tensor(out=rm[:, :hsz, 30:60], in0=ev,
                                        in1=t1v, op=Alu.subtract)
                dch = sp.tile([128, CHK, 60], BF16, tag="dch")
                nc.scalar.activation(out=dch[:, :hsz], in_=rm[:, :hsz],
                                     func=Act.Sin,
                                     scale=float(2.0 * np.pi))
                for q0 in range(0, hsz, NSPT):
                    dps = psB2.tile([60, NSPT * 128], BF16, tag="dps", bufs=1)
                    for s4 in range(NSPT):
                        nc.tensor.transpose(
                            out=dps[:, s4 * 128:(s4 + 1) * 128],
                            in_=dch[:, q0 + s4, :], identity=ident[:])
                    tt = (g0 + q0) // NSPT
                    dtile = sp.tile([60, TE], BF16, tag="disT", bufs=8,
                                    name=f"disT{tt}")
                    disT_tiles[tt] = dtile
                    nc.vector.tensor_copy(out=dtile[:], in_=dps[:])

            with (
                tc.tile_pool(name="ps_mm", bufs=2, space="PSUM") as ps_mm,
                tc.tile_pool(name="ps_agg", bufs=2, space="PSUM") as ps_agg,
                tc.tile_pool(name="ps_B2", bufs=2, space="PSUM") as psB2,
            ):
                agg_ps = {}
                for t in range(n_tiles):
                    if t % 4 == 0:
                        emit_dis_chunk(t // 4)
                    if USE_GATHER1:
                        qT = sp.tile([128, 2, 2 * TE], BF16, tag="qT", bufs=3)
                        nc.gpsimd.dma_gather(
                            out_ap=qT[:], in_ap=q_dram[:],
                            idxs_ap=qgi_s[:, t, :],
                            num_idxs=2 * TE, num_idxs_reg=2 * TE,
                            elem_size=HID, transpose=True)
                        q_lo, q_hi = qT[:, :, 0:TE], qT[:, :, TE:2 * TE]
                    else:
                        qTa = sp.tile([128, 2, TE], BF16, tag="qTa", bufs=3)
                        nc.gpsimd.dma_gather(
                            out_ap=qTa[:], in_ap=q_dram[:],
                            idxs_ap=qgi_s[:, t, 0:S_IDX],
                            num_idxs=TE, num_idxs_reg=TE,
                            elem_size=HID, transpose=True)
                        qTb = sp.tile([128, 2, TE], BF16, tag="qTb", bufs=3)
                        nc.gpsimd.dma_gather(
                            out_ap=qTb[:], in_ap=q_dram[:],
                            idxs_ap=qgi_s[:, t, S_IDX:2 * S_IDX],
                            num_idxs=TE, num_idxs_reg=TE,
                            elem_size=HID, transpose=True)
                        q_lo, q_hi = qTa[:], qTb[:]
                    sctT_s = sp.tile([128, NSPT, 128], BF16, tag="sctT")
                    nc.sync.dma_start(
                        out=sctT_s[:],
                        in_=sctT_d[:, t * NSPT:(t + 1) * NSPT, :])
                    sct_s = sp.tile([128, NSPT, 128],
                                    F8 if USE_FP8_SCT else BF16, tag="sct")
                    nc.scalar.dma_start(
                        out=sct_s[:],
                        in_=sct_d[:, t * NSPT:(t + 1) * NSPT, :])

                    dT = disT_tiles[t][:]
                    # Q + lat merged on (otherwise idle) DVE: one fewer
                    # PE identity pass per o-chunk.
                    qlT = sp.tile([128, 2, TE], BF16, tag="qlT")
                    nc.vector.tensor_tensor(
                        out=qlT[:], in0=q_lo, in1=q_hi, op=Alu.add)
                    # layer 1 (feature-major); both o-chunks in one psum
                    # tile so silu1 is a single wide ACT call.
                    x1T = sp.tile([128, 2, TE],
                                  F8 if USE_FP8_L2 else BF16, tag="x1T")
                    p1 = ps_mm.tile([128, 2 * TE], F32, tag="mm", name=f"p1_{t}")
                    for oc in range(2):
                        sl1 = slice(oc * TE, (oc + 1) * TE)
                        nc.tensor.matmul(out=p1[:, sl1], lhsT=ident[:],
                                         rhs=qlT[:, oc, :], start=True, stop=False)
                        for s in range(NSPT):
                            w = int(sub2w[t * NSPT + s])
                            nc.tensor.matmul(
                                out=p1[:, oc * TE + s * 128:oc * TE + (s + 1) * 128],
                                lhsT=P_sb[:, w, oc * 128:(oc + 1) * 128],
                                rhs=sctT_s[:, s, :], start=False, stop=False)
                        nc.tensor.matmul(out=p1[:, sl1],
                                         lhsT=Wd_s[:, oc * 128:(oc + 1) * 128],
                                         rhs=dT, start=False, stop=True)
                    nc.scalar.activation(
                        out=x1T[:].rearrange("p c e -> p (c e)"), in_=p1[:],
                        func=Act.Silu)

                    # layer 2 (operand-flipped -> edge-major) + ones column
                    x2_ps = ps_mm.tile([128, NSPT * HID], F32, tag="mm",
                                       name=f"x2ps_{t}")
                    x2 = sp.tile([128, NSPT, HID + 1],
                                 F8 if USE_FP8_SCT else BF16, tag="x2s")
                    for s in range(NSPT):
                        osl = s * HID
                        nc.tensor.matmul(out=x2_ps[:, osl:osl + HID],
                                         lhsT=ones_row[:], rhs=be2_s[:],
                                         start=True, stop=False)
                        if USE_FP8_L2:
                            nc.tensor.matmul(
                                out=x2_ps[:, osl:osl + HID],
                                lhsT=x1T[:, :, s * 128:(s + 1) * 128],
                                rhs=W2_s[:],
                                perf_mode=mybir.MatmulPerfMode.DoubleRow,
                                start=False, stop=True)
                        else:
                            for kc in range(2):
                                nc.tensor.matmul(
                                    out=x2_ps[:, osl:osl + HID],
                                    lhsT=x1T[:, kc, s * 128:(s + 1) * 128],
                                    rhs=W2_s[:, kc, :], start=False,
                                    stop=(kc == 1))
                    nc.scalar.activation(
                        out=x2[:, :, 0:HID],
                        in_=x2_ps[:].rearrange("p (s h) -> p s h", s=NSPT),
                        func=Act.Silu)
                    nc.vector.tensor_copy(out=x2[:, :, HID], in_=ones4[:])

                    # scatter into per-window PSUM accumulators
                    # (DoubleRow pairs two subtiles when both are in the
                    # same window; odd/straddling subtiles go singly)
                    s = 0
                    while s < NSPT:
                        g = t * NSPT + s
                        w = int(sub2w[g])
                        first, last = g == wstart[w], g == wstart[w + 1] - 1
                        pair = (s + 1 < NSPT and not last
                                and int(sub2w[g + 1]) == w)
                        if first:
                            agg_ps[w] = ps_agg.tile([128, HID + 1], F32,
                                                    tag="agg", name=f"agg{w}")
                        if pair and USE_FP8_SCT:
                            last = g + 1 == wstart[w + 1] - 1
                            nc.tensor.matmul(
                                out=agg_ps[w][:],
                                lhsT=sct_s[:, s:s + 2, :],
                                rhs=x2[:, s:s + 2, :],
                                perf_mode=mybir.MatmulPerfMode.DoubleRow,
                                start=first, stop=last,
                                skip_group_check=True)
                            s += 2
                        else:
                            nc.tensor.matmul(out=agg_ps[w][:],
                                             lhsT=sct_s[:, s, :],
                                             rhs=x2[:, s, :], start=first,
                                             stop=last,
                                             skip_group_check=True)
                            s += 1
                        if last:
                            ap = agg_ps.pop(w)
                            cm = sp.tile([128, 1], F32, tag="cnt")
                            nc.vector.tensor_scalar(
                                out=cm[:], in0=ap[:, HID:HID + 1], scalar1=1.0,
                                scalar2=None, op0=Alu.max)
                            nc.vector.reciprocal(out=inv_cnt[:, w:w + 1], in_=cm[:])
                            nc.vector.tensor_tensor(
                                out=agg_bf[:, w, :], in0=ap[:, 0:HID],
                                in1=inv_cnt[:, w:w + 1].to_broadcast([128, HID]),
                                op=Alu.mult)
                            for c2 in range(2):
                                tp = psB2.tile([128, 128], BF16, tag="aggT", bufs=1,
                                               name=f"aggT{w}_{c2}")
                                nc.tensor.transpose(
                                    out=tp[:],
                                    in_=agg_bf[:, w, c2 * 128:(c2 + 1) * 128],
                                    identity=ident[:])
                                nc.vector.tensor_copy(
                                    out=aggT[:, c2, w * 128:(w + 1) * 128],
                                    in_=tp[:])

            # ---------------- phase D: node MLP ----------------
            with tc.tile_pool(name="psD", bufs=1, space="PSUM") as psD:
                h1T = pp.tile([128, 2, NLOC], BF16)
                for oc in range(2):
                    hp = psD.tile([128, NLOC], F32, tag="h1T", bufs=2)
                    for kc in range(4):
                        rhs = nfT_loc_s[:, kc, :] if kc < 2 else aggT[:, kc - 2, :]
                        for nsp in range(0, NLOC, 512):
                            ln = min(512, NLOC - nsp)
                            nc.tensor.matmul(
                                out=hp[:, nsp:nsp + ln],
                                lhsT=Wn1_s[:, kc, oc * 128:(oc + 1) * 128],
                                rhs=rhs[:, nsp:nsp + ln],
                                start=(kc == 0), stop=(kc == 3),
                                skip_group_check=True)
                    nc.scalar.activation(out=h1T[:, oc, :], in_=hp[:],
                                         func=Act.Silu, bias=bn1_s[:, oc:oc + 1])
                outs = pp.tile([128, NW, HID], F32)
                for w in range(NW):
                    op = psD.tile([128, HID], F32, tag="outp", bufs=2)
                    nc.tensor.matmul(out=op[:], lhsT=ones_row[:], rhs=bn2_s[:],
                                     start=True, stop=False)
                    for kc in range(2):
                        nc.tensor.matmul(out=op[:],
                                         lhsT=h1T[:, kc, w * 128:(w + 1) * 128],
                                         rhs=Wn2_s[:, kc, :],
                                         start=False, stop=(kc == 1))
                    o2 = sp.tile([128, HID], F32, tag="o2")
                    nc.scalar.activation(out=o2[:], in_=op[:], func=Act.Silu)
                    nc.vector.tensor_tensor(out=outs[:, w, :], in0=o2[:],
                                            in1=nf_loc_s[:, w, :], op=Alu.add)
                nc.sync.dma_start(out=out_d[:], in_=outs[:])

    return nc


# --------------------------------------------------------------------------
# entry point
# --------------------------------------------------------------------------

def kernel(**inputs):
    from concourse.bass_utils import run_bass_kernel_spmd

    in_maps, meta = _prep(inputs)
    nc = build_program(meta)
    nc.compile()
    res = run_bass_kernel_spmd(nc, in_maps, core_ids=list(range(N_CORES)))
    out = np.zeros((N_NODES, HID), np.float32)
    for c in range(N_CORES):
        o = np.asarray(res.results[c]["out"], np.float32)  # [128, NW, HID]
        o = o.transpose(1, 0, 2).reshape(NLOC, HID)
        out[c * NPC:(c + 1) * NPC] = o[:NPC]
    return out



# revision 9
# speedup vs baseline: 1.4130x; 1.4130x over previous
"""E3GNN message-passing layer on 8 TRN2 NeuronCores (Bass/Tile).

Strategy (edge data parallelism by *source-node range*):
  - Sort edges by src; core i owns src in [1250*i, 1250*(i+1)).  Each core
    fully aggregates its own nodes' messages, so NO cross-core collective is
    needed: each core returns out[1250*i : 1250*(i+1)] and the host stacks.
  - Per-edge layer-1 input is decomposed:  e_in @ We1 =
        P[src] + Q[dst] + latproj[g] + dis @ Wd   (+ be1 folded into P)
    with P = nf @ We1[0:256] + be1 (local rows only), Q = nf @ We1[256:512]
    (all rows, computed on every core), latproj = lat_ips @ We1[512:521],
    Wd = We1[521:581].  P/Q/lat tables are built on device in bf16.
  - Q and latproj rows are fetched per edge with gpsimd.dma_gather
    (transpose=True) -> feature-major tiles that accumulate into PSUM via
    identity matmuls.  P[src] is expanded via host-provided one-hot
    (edges sorted => src spans one 128-node window per 128-edge subtile).
  - Layer 2 swaps matmul operand roles (lhsT = x1^T tile) so x2 lands
    edge-major for the scatter, with zero explicit transposes.
  - Scatter-mean: one-hot^T @ [x2 | 1] accumulated in PSUM per 128-node
    window (sorted edges => sequential windows), then the node MLP.
Compute dtype bf16 (fp32 PSUM accumulation); layer 2 and the scatter
matmul run in fp8-e4m3 DoubleRow (2x PE) - rel err stays ~2.4e-4, far
under the 2e-2 gate.  Sinusoid range reduction uses the 2^23 magic
constant (RNE round) since ScalarE Sin only covers [-pi, pi].
Note: gpsimd dma_gather with num_idxs=1024 crashes the NEFF on silicon
(NRT_EXEC_UNIT_UNRECOVERABLE); keep num_idxs <= 512 per call.
"""

import os
import sys

import numpy as np

sys.path.insert(0, "/opt/trn_rl_repo")

import ml_dtypes  # noqa: E402

import concourse.bass as bass  # noqa: E402
import concourse.mybir as mybir  # noqa: E402
import concourse.tile as tile  # noqa: E402
from concourse import bacc, library_config  # noqa: E402
from concourse.masks import make_identity  # noqa: E402

# ---- problem constants (hardcoded per contract) ----
N_NODES = 10000
N_EDGES = 200000
N_GRAPHS = 500
HID = 256
NUM_FREQS = 10
N_CORES = 8
NPC = N_NODES // N_CORES          # nodes per core = 1250
WSZ = 128                         # node-window size
NW = 10                           # windows per core (1280 slots, 30 unused)
NLOC = NW * WSZ                   # 1280 local node slots
TE = 512                          # edges per pipeline tile
NSPT = TE // 128                  # 4 subtiles per tile
G_PAD = 512                       # lattice table rows (500 real)
USE_FP8_L2 = os.environ.get("K_FP8_L2", "1") == "1"
USE_FP8_SCT = os.environ.get("K_FP8_SCT", "1") == "1"
USE_GATHER1 = os.environ.get("K_GATHER1", "0") == "1"

F32 = mybir.dt.float32
F8 = mybir.dt.float8e4
BF16 = mybir.dt.bfloat16
I16 = mybir.dt.int16
BF = ml_dtypes.bfloat16
F8NP = ml_dtypes.float8_e4m3fn
ALU = None  # set after import in functions


# --------------------------------------------------------------------------
# single-blob input layout
#
# The axon launch path charges ~40us per bound tensor per execution plus
# ~20us per MB shipped, so all inputs ride ONE int16 DRAM blob.  Each
# entry is a full [p0:p1, off:off+ln] rectangle (i16 units per
# partition); f32/bf16 payloads are bitcast views of the i16 storage.
# One-hot scatter matrices are NOT shipped (26 B/edge on the wire) --
# they are rebuilt on device from `srel` with an iota compare, so the
# wire cost per edge is 4 B.
# --------------------------------------------------------------------------

def _layout(n_sub, n_tiles):
    L, off = {}, [0]

    def add(name, rows, ln):
        L[name] = (rows[0], rows[1], off[0], ln)
        off[0] += ln

    add("nfT", (0, 128), 2 * 10240)        # bf16 [128,2,10240]
    add("nfT_loc", (0, 128), 2 * NLOC)     # bf16 [128,2,NLOC]
    add("W1a", (0, 128), 2 * HID)          # bf16 [128,2,HID]
    add("W1b", (0, 128), 2 * HID)
    add("W2", (0, 128), 2 * HID)
    add("Wn1", (0, 128), 4 * HID)
    add("Wn2", (0, 128), 2 * HID)
    # shared 256-column rectangle for sub-128-partition constants (bf16)
    add("misc", (0, 128), HID)             # Wd rows 0:60, Wlat 64:73,
    #                                        be1/be2/bn2 rows 80/81/82
    add("latw", (0, 128), 2 * 4 * 9)       # f32 [128,4,9]
    add("bn1", (0, 128), 2 * 2)            # f32 [128,2]
    add("freqs", (0, 128), 2 * NUM_FREQS)  # f32 [128,10]
    add("iota", (0, 128), 2 * 128)         # f32 [128,128] row 0..127
    add("nf_loc", (0, 128), 2 * NW * HID)  # f32 [128,NW,HID]
    add("frac_s", (0, 128), 2 * n_sub * 3)
    add("frac_d", (0, 128), 2 * n_sub * 3)
    add("srel", (0, 128), 2 * n_sub)       # f32, -1 on padding
    add("qg_idx", (0, 128), n_tiles * 2 * (TE // 16))  # i16
    return L, off[0]


# --------------------------------------------------------------------------
# host-side sharding / index prep (pure indexing, no FP math on data)
# --------------------------------------------------------------------------

def _prep(inputs):
    nf = np.asarray(inputs["node_features"], np.float32)
    frac = np.asarray(inputs["frac_coords"], np.float32)
    lat = np.asarray(inputs["lattices"], np.float32)
    ei = np.asarray(inputs["edge_index"], np.int64)
    e2g = np.asarray(inputs["edge2graph"], np.int64)
    src, dst = ei[0], ei[1]

    core = src // NPC
    loc = src - core * NPC
    win = loc // WSZ
    order = np.lexsort((e2g, win, core))  # sort by (core, window, graph)
    src_s, dst_s, g_s = src[order], dst[order], e2g[order]
    core_s, loc_s, win_s = core[order], loc[order], win[order]

    # per (core, window) edge counts -> shared static subtile counts
    cnts = np.zeros((N_CORES, NW), np.int64)
    for c in range(N_CORES):
        cw = win_s[core_s == c]
        for w in range(NW):
            cnts[c, w] = int((cw == w).sum())
    sub_w = np.maximum(1, -(-cnts.max(axis=0) // 128))  # ceil, >=1
    n_sub = int(sub_w.sum())
    n_sub = -(-n_sub // NSPT) * NSPT                    # round to tile
    sub_w[NW - 1] += n_sub - int(sub_w.sum())
    e_pad = n_sub * 128
    n_tiles = n_sub // NSPT
    wstart = np.concatenate(([0], np.cumsum(sub_w)))    # window->subtile range

    per_core = []
    for c in range(N_CORES):
        m = core_s == c
        csrc_rel = (loc_s[m] - win_s[m] * WSZ)
        cdst, cg, cwin = dst_s[m], g_s[m], win_s[m]
        dsti = np.zeros(e_pad, np.int64)
        gi = np.zeros(e_pad, np.int64)
        srel = np.full(e_pad, -1, np.int64)
        fsrc = np.zeros((e_pad, 3), np.float32)
        fdst = np.zeros((e_pad, 3), np.float32)
        for w in range(NW):
            wm = cwin == w
            k = int(wm.sum())
            s0 = wstart[w] * 128
            assert k <= (wstart[w + 1] - wstart[w]) * 128
            sl = slice(s0, s0 + k)
            dsti[sl] = cdst[wm]
            gi[sl] = cg[wm]
            srel[sl] = csrc_rel[wm]
            esrc = c * NPC + w * WSZ + csrc_rel[wm]
            fsrc[sl] = frac[esrc]
            fdst[sl] = frac[cdst[wm]]

        def wrap_idx(idx, per_call):
            # dma_gather idx layout: element t of a call lives at
            # [t % 16, t // 16]; replicate the 16-row block to 128 partitions.
            b = idx.reshape(n_tiles, per_call // 16, 16).transpose(0, 2, 1)
            b = np.ascontiguousarray(b.transpose(1, 0, 2)).astype(np.int16)
            return np.tile(b, (8, 1, 1))  # [128, n_tiles, per_call//16]

        ew = lambda a: np.ascontiguousarray(
            a.reshape(n_sub, 128, -1).transpose(1, 0, 2)).astype(np.float32)

        qg = np.concatenate(
            [dsti.reshape(n_tiles, TE), (gi + 10240).reshape(n_tiles, TE)],
            axis=1).reshape(-1)  # per tile: [dst(512) | 10240+g(512)]
        per_core.append(dict(
            qg_idx=wrap_idx(qg, 2 * TE),
            srel=ew(srel.astype(np.float32).reshape(e_pad, 1))[:, :, 0],
            frac_s=ew(fsrc), frac_d=ew(fdst),
        ))

    # ---- shared (replicated) payloads ----
    nf_pad = np.zeros((10240, HID), np.float32)
    nf_pad[:N_NODES] = nf
    nfT_w = np.ascontiguousarray(
        nf_pad.reshape(10240, 2, 128).transpose(2, 1, 0)).astype(np.float32)
    lat_flat = np.zeros((G_PAD, 9), np.float32)
    lat_flat[:N_GRAPHS] = lat.reshape(N_GRAPHS, 9)
    latw = np.ascontiguousarray(
        lat_flat.reshape(4, 128, 9).transpose(1, 0, 2)).astype(np.float32)

    We1 = np.asarray(inputs["We1"], np.float32)
    wshape = lambda w, k: np.ascontiguousarray(
        w.reshape(k, 128, HID).transpose(1, 0, 2)).astype(np.float32)
    misc = np.zeros((128, HID), np.float32)
    misc[0:60] = We1[521:581]                                   # Wd
    misc[64:73] = We1[512:521]                                  # Wlat
    misc[80] = np.asarray(inputs["be1"], np.float32)
    misc[81] = np.asarray(inputs["be2"], np.float32)
    misc[82] = np.asarray(inputs["bn2"], np.float32)
    shared = dict(
        nfT=nfT_w, latw=latw,
        W1a=wshape(We1[0:256], 2), W1b=wshape(We1[256:512], 2),
        W2=wshape(np.asarray(inputs["We2"], np.float32), 2),
        Wn1=wshape(np.asarray(inputs["Wn1"], np.float32), 4),
        Wn2=wshape(np.asarray(inputs["Wn2"], np.float32), 2),
        misc=misc,
        bn1=np.ascontiguousarray(
            np.asarray(inputs["bn1"], np.float32).reshape(2, 128).T),
        freqs=np.tile(np.arange(NUM_FREQS, dtype=np.float32)
                      .reshape(1, NUM_FREQS), (128, 1)),
        iota=np.tile(np.arange(128, dtype=np.float32).reshape(1, 128),
                     (128, 1)),
    )

    L, TOT = _layout(n_sub, n_tiles)
    BF16_KEYS = {"nfT", "nfT_loc", "W1a", "W1b", "W2", "Wn1", "Wn2", "misc"}

    def pack(m):
        blob = np.zeros((128, TOT), np.int16)
        for name, arr in m.items():
            p0, p1, off, ln = L[name]
            if name == "qg_idx":
                pay = arr.reshape(p1 - p0, -1)
            elif name in BF16_KEYS:
                pay = arr.astype(BF).view(np.int16).reshape(p1 - p0, -1)
            else:
                pay = arr.astype(np.float32).view(np.int16).reshape(
                    p1 - p0, -1)
            assert pay.shape[1] == ln, (name, pay.shape, ln)
            blob[p0:p1, off:off + ln] = pay
        return blob

    in_maps = []
    for c in range(N_CORES):
        m = dict(shared)
        m.update(per_core[c])
        m["nfT_loc"] = np.ascontiguousarray(nfT_w[:, :, c * NPC:c * NPC + NLOC])
        nl = np.zeros((NLOC, HID), np.float32)
        nl[:NPC] = nf[c * NPC:(c + 1) * NPC]
        m["nf_loc"] = np.ascontiguousarray(
            nl.reshape(NW, 128, HID).transpose(1, 0, 2))
        in_maps.append({"blob": pack(m)})

    meta = dict(n_sub=n_sub, n_tiles=n_tiles, wstart=[int(x) for x in wstart])
    return in_maps, meta


# --------------------------------------------------------------------------
# device program
# --------------------------------------------------------------------------

def build_program(meta):
    n_sub, n_tiles = meta["n_sub"], meta["n_tiles"]
    wstart = meta["wstart"]
    sub2w = np.zeros(n_sub, np.int64)
    for w in range(NW):
        sub2w[wstart[w]:wstart[w + 1]] = w
    S_IDX = TE // 16
    Alu = mybir.AluOpType
    Act = mybir.ActivationFunctionType

    nc = bacc.Bacc("TRN2", target_bir_lowering=False, debug=False)

    L, TOT = _layout(n_sub, n_tiles)
    blob = nc.declare_dram_parameter("blob", [128, TOT], I16, isOutput=False)

    def bview(name, dt, shape=None):
        """Blob rectangle as a typed DRAM view [p0:p1, ...shape]."""
        p0, p1, off, ln = L[name]
        v = blob[p0:p1, off:off + ln]
        if dt != I16:
            v = v.bitcast(dt)
        if shape is not None:
            lead = "p (" + " ".join(f"d{i}" for i in range(len(shape))) + ")"
            tail = " ".join(f"d{i}" for i in range(len(shape)))
            v = v.rearrange(f"{lead} -> p {tail}",
                            **{f"d{i}": s for i, s in enumerate(shape)})
        return v

    nfT = bview("nfT", BF16, (2, 10240))
    nfT_loc = bview("nfT_loc", BF16, (2, NLOC))
    nf_loc = bview("nf_loc", F32, (NW, HID))
    latw = bview("latw", F32, (4, 9))
    W1a = bview("W1a", BF16, (2, HID)); W1b = bview("W1b", BF16, (2, HID))
    misc = bview("misc", BF16)        # Wd 0:60, Wlat 64:73, be1/be2/bn2 80/81/82
    W2 = bview("W2", BF16, (2, HID))
    Wn1 = bview("Wn1", BF16, (4, HID)); Wn2 = bview("Wn2", BF16, (2, HID))
    bn1 = bview("bn1", F32)
    freqs = bview("freqs", F32)
    iota_d = bview("iota", F32)
    qg_idx = bview("qg_idx", I16, (n_tiles, 2 * S_IDX))
    frac_s = bview("frac_s", F32, (n_sub, 3))
    frac_d = bview("frac_d", F32, (n_sub, 3))
    srel_d = bview("srel", F32)
    out_d = nc.declare_dram_parameter("out", [128, NW, HID], F32, isOutput=True)

    q_dram = nc.dram_tensor("q_dram", [10240 + G_PAD, HID], BF16)

    with tile.TileContext(nc) as tc:
        nc.gpsimd.load_library(library_config.mlp)

        with (
            tc.tile_pool(name="persist", bufs=1) as pp,
            tc.tile_pool(name="stage", bufs=2) as sp,
        ):
            # ---------------- phase A: constants into SBUF ----------------
            ident = pp.tile([128, 128], BF16)
            make_identity(nc, ident[:])
            ones_row = pp.tile([1, 128], BF16)
            nc.gpsimd.memset(ones_row[:], 1.0)
            ones4 = pp.tile([128, NSPT], F8 if USE_FP8_SCT else BF16)
            nc.gpsimd.memset(ones4[:], 1.0)

            def load(dram_view, shape, name, dt=BF16):
                t = pp.tile(list(shape), dt, name=name, tag=name)
                nc.sync.dma_start(out=t[:], in_=dram_view)
                return t

            W1a_s = load(W1a[:], (128, 2, HID), "W1a_s")
            W1b_s = load(W1b[:], (128, 2, HID), "W1b_s")
            Wlat_s = load(misc[64:73, :], (9, HID), "Wlat_s")
            Wd_s = load(misc[0:60, :], (60, HID), "Wd_s")
            Wn1_s = load(Wn1[:], (128, 4, HID), "Wn1_s")
            Wn2_s = load(Wn2[:], (128, 2, HID), "Wn2_s")
            be1_s = load(misc[80:81, :], (1, HID), "be1_s")
            be2_s = load(misc[81:82, :], (1, HID), "be2_s")
            bn2_s = load(misc[82:83, :], (1, HID), "bn2_s")
            if USE_FP8_L2:
                W2_bf = load(W2[:], (128, 2, HID), "W2_bf")
                W2_s = pp.tile([128, 2, HID], F8, name="W2_s", tag="W2_s")
                nc.vector.tensor_copy(out=W2_s[:], in_=W2_bf[:])
            else:
                W2_s = load(W2[:], (128, 2, HID), "W2_s")
            bn1_s = pp.tile([128, 2], F32)
            nc.sync.dma_start(out=bn1_s[:], in_=bn1[:])
            freqs_s = pp.tile([128, NUM_FREQS], F32)
            nc.sync.dma_start(out=freqs_s[:], in_=freqs[:])
            iota_s = pp.tile([128, 128], F32)
            nc.sync.dma_start(out=iota_s[:], in_=iota_d[:])
            srel_s = pp.tile([128, n_sub], F32)
            nc.sync.dma_start(out=srel_s[:], in_=srel_d[:])
            nfT_loc_s = load(nfT_loc[:], (128, 2, NLOC), "nfT_loc_s")
            nf_loc_s = pp.tile([128, NW, HID], F32)
            nc.sync.dma_start(out=nf_loc_s[:], in_=nf_loc[:])

            # ---------------- phase B: node/graph tables ----------------
            P_sb = pp.tile([128, NW, HID], BF16)
            with tc.tile_pool(name="psB", bufs=2, space="PSUM") as psB:
                for w in range(NW):
                    ps = psB.tile([128, HID], F32, tag="tbl", bufs=4)
                    nc.tensor.matmul(out=ps[:], lhsT=ones_row[:],
                                     rhs=be1_s[:], start=True, stop=False)
                    for kc in range(2):
                        nc.tensor.matmul(
                            out=ps[:],
                            lhsT=nfT_loc_s[:, kc, w * 128:(w + 1) * 128],
                            rhs=W1a_s[:, kc, :], start=False, stop=(kc == 1))
                    nc.vector.tensor_copy(out=P_sb[:, w, :], in_=ps[:])

                with tc.tile_pool(name="nfull", bufs=1) as nfp:
                    NCH = 8
                    nfT_ch = []
                    for ci in range(NCH):
                        nft = nfp.tile([128, 2, 10240 // NCH], BF16,
                                       name=f"nfT{ci}", tag=f"nfT{ci}")
                        nc.gpsimd.dma_start(
                            out=nft[:],
                            in_=nfT[:, :, ci * 1280:(ci + 1) * 1280])
                        nfT_ch.append(nft)
                    for w in range(80):
                        ci, wl = w // 10, w % 10
                        ps = psB.tile([128, HID], F32, tag="tbl", bufs=4)
                        for kc in range(2):
                            nc.tensor.matmul(
                                out=ps[:],
                                lhsT=nfT_ch[ci][:, kc, wl * 128:(wl + 1) * 128],
                                rhs=W1b_s[:, kc, :], start=(kc == 0), stop=(kc == 1))
                        qs = sp.tile([128, HID], BF16, tag="qtile", bufs=8)
                        if w % 4 == 3:
                            nc.scalar.copy(out=qs[:], in_=ps[:])
                        else:
                            nc.vector.tensor_copy(out=qs[:], in_=ps[:])
                        eng = nc.sync if w % 2 == 0 else nc.scalar
                        eng.dma_start(out=q_dram[w * 128:(w + 1) * 128, :],
                                      in_=qs[:])

                # lattice inner products + projection table
                latc = pp.tile([128, 4, 9], F32)
                nc.sync.dma_start(out=latc[:], in_=latw[:])
                ipT_ps = psB.tile([9, 512], BF16, tag="ipT")
                for c4 in range(4):
                    a1 = sp.tile([128, 3, 3, 3], F32, tag="lat1")
                    a2 = sp.tile([128, 3, 3, 3], F32, tag="lat2")
                    lv = latc[:, c4, :].rearrange("p (i j) -> p i j", i=3)
                    nc.vector.tensor_copy(
                        out=a1[:], in_=lv.unsqueeze(2).to_broadcast([128, 3, 3, 3]))
                    nc.vector.tensor_copy(
                        out=a2[:], in_=lv.unsqueeze(1).to_broadcast([128, 3, 3, 3]))
                    nc.vector.tensor_tensor(out=a1[:], in0=a1[:], in1=a2[:],
                                            op=Alu.mult)
                    ip = sp.tile([128, 9], F32, tag="lat3")
                    pr = a1[:].rearrange("p i k j -> p (i k) j")
                    nc.vector.tensor_tensor(out=ip[:], in0=pr[:, :, 0],
                                            in1=pr[:, :, 1], op=Alu.add)
                    nc.vector.tensor_tensor(out=ip[:], in0=ip[:],
                                            in1=pr[:, :, 2], op=Alu.add)
                    ipb = sp.tile([128, 9], BF16, tag="lat4")
                    nc.vector.tensor_copy(out=ipb[:], in_=ip[:])
                    nc.tensor.transpose(out=ipT_ps[:, c4 * 128:(c4 + 1) * 128],
                                        in_=ipb[:], identity=ident[:])
                ipT_s = pp.tile([9, 512], BF16)
                nc.vector.tensor_copy(out=ipT_s[:], in_=ipT_ps[:])
                for c4 in range(4):
                    ps = psB.tile([128, HID], F32, tag="tbl", bufs=4)
                    nc.tensor.matmul(out=ps[:],
                                     lhsT=ipT_s[:, c4 * 128:(c4 + 1) * 128],
                                     rhs=Wlat_s[:], start=True, stop=True)
                    ls = sp.tile([128, HID], BF16, tag="qtile", bufs=8)
                    nc.vector.tensor_copy(out=ls[:], in_=ps[:])
                    nc.sync.dma_start(
                        out=q_dram[10240 + c4 * 128:10240 + (c4 + 1) * 128, :],
                        in_=ls[:])

            # ---------------- phase B2 prep: frac diff (whole) ----------------
            disT_tiles = {}
            diff = pp.tile([128, n_sub, 3], F32)
            with tc.tile_pool(name="fracp", bufs=1) as fp:
                fs = fp.tile([128, n_sub, 3], F32)
                fd = fp.tile([128, n_sub, 3], F32)
                nc.sync.dma_start(out=fs[:], in_=frac_s[:])
                nc.sync.dma_start(out=fd[:], in_=frac_d[:])
                neg = fp.tile([128, n_sub, 3], F32)
                nc.vector.tensor_tensor(out=diff[:], in0=fd[:], in1=fs[:],
                                        op=Alu.subtract)
                # floor-mod into [0,1): x += (x < 0)
                nc.vector.tensor_scalar(out=neg[:], in0=diff[:], scalar1=0.0,
                                        scalar2=None, op0=Alu.is_lt)
                nc.vector.tensor_tensor(out=diff[:], in0=diff[:], in1=neg[:],
                                        op=Alu.add)

            # ---------------- phase C: edge pipeline (B2 inlined) -------------
            qgi_s = pp.tile([128, n_tiles, 2 * S_IDX], I16)
            nc.sync.dma_start(out=qgi_s[:], in_=qg_idx[:])

            agg_bf = pp.tile([128, NW, HID], BF16)
            aggT = pp.tile([128, 2, NLOC], BF16)
            inv_cnt = pp.tile([128, NW], F32)

            CHK = 16  # subtiles per dis-chunk = 4 edge tiles
            MAGIC = 12582912.0  # 1.5 * 2**23

            def emit_dis_chunk(k):
                # sinusoid embeddings for subtiles [k*CHK, k*CHK+hsz),
                # written straight into disT_tiles (feature-major).
                g0 = k * CHK
                hsz = min(CHK, n_sub - g0)
                sl = slice(g0, g0 + hsz)
                emb = sp.tile([128, CHK, 3, NUM_FREQS], F32, tag="emb")
                nc.vector.tensor_tensor(
                    out=emb[:, :hsz],
                    in0=diff[:, sl, :].unsqueeze(3).to_broadcast(
                        [128, hsz, 3, NUM_FREQS]),
                    in1=freqs_s[:].unsqueeze(1).unsqueeze(1).to_broadcast(
                        [128, hsz, 3, NUM_FREQS]),
                    op=Alu.mult)
                # range-reduce via the 2^23 magic constant (RNE round):
                # r = y - round(y) in [-.5,.5]; sin(2pi*y) = Sin(2pi*r);
                # cos(2pi*y) = sin(2pi*(y+0.25)) the same way.
                ev = emb[:, :hsz].rearrange("p s c k -> p s (c k)")
                rm = sp.tile([128, CHK, 60], F32, tag="embm")
                t1 = sp.tile([128, CHK, 30], F32, tag="embt")
                t1v = t1[:, :hsz]
                nc.vector.tensor_scalar(out=t1v, in0=ev, scalar1=MAGIC,
                                        scalar2=None, op0=Alu.add)
                nc.vector.tensor_scalar(out=t1v, in0=t1v, scalar1=MAGIC,
                                        scalar2=None, op0=Alu.subtract)
                nc.vector.tensor_tensor(out=rm[:, :hsz, 0:30], in0=ev,
                                        in1=t1v, op=Alu.subtract)
                nc.vector.tensor_scalar(out=t1v, in0=ev, scalar1=0.25,
                                        scalar2=MAGIC, op0=Alu.add,
                                        op1=Alu.add)
                nc.vector.tensor_scalar(out=t1v, in0=t1v, scalar1=MAGIC,
                                        scalar2=0.25, op0=Alu.subtract,
                                        op1=Alu.subtract)
                nc.vector.tensor_tensor(out=rm[:, :hsz, 30:60], in0=ev,
                                        in1=t1v, op=Alu.subtract)
                dch = sp.tile([128, CHK, 60], BF16, tag="dch")
                nc.scalar.activation(out=dch[:, :hsz], in_=rm[:, :hsz],
                                     func=Act.Sin,
                                     scale=float(2.0 * np.pi))
                for q0 in range(0, hsz, NSPT):
                    dps = psB2.tile([60, NSPT * 128], BF16, tag="dps", bufs=1)
                    for s4 in range(NSPT):
                        nc.tensor.transpose(
                            out=dps[:, s4 * 128:(s4 + 1) * 128],
                            in_=dch[:, q0 + s4, :], identity=ident[:])
                    tt = (g0 + q0) // NSPT
                    dtile = sp.tile([60, TE], BF16, tag="disT", bufs=8,
                                    name=f"disT{tt}")
                    disT_tiles[tt] = dtile
                    nc.vector.tensor_copy(out=dtile[:], in_=dps[:])

            with (
                tc.tile_pool(name="ps_mm", bufs=2, space="PSUM") as ps_mm,
                tc.tile_pool(name="ps_agg", bufs=2, space="PSUM") as ps_agg,
                tc.tile_pool(name="ps_B2", bufs=2, space="PSUM") as psB2,
            ):
                agg_ps = {}
                for t in range(n_tiles):
                    if t % 4 == 0:
                        emit_dis_chunk(t // 4)
                    if USE_GATHER1:
                        qT = sp.tile([128, 2, 2 * TE], BF16, tag="qT", bufs=3)
                        nc.gpsimd.dma_gather(
                            out_ap=qT[:], in_ap=q_dram[:],
                            idxs_ap=qgi_s[:, t, :],
                            num_idxs=2 * TE, num_idxs_reg=2 * TE,
                            elem_size=HID, transpose=True)
                        q_lo, q_hi = qT[:, :, 0:TE], qT[:, :, TE:2 * TE]
                    else:
                        qTa = sp.tile([128, 2, TE], BF16, tag="qTa", bufs=3)
                        nc.gpsimd.dma_gather(
                            out_ap=qTa[:], in_ap=q_dram[:],
                            idxs_ap=qgi_s[:, t, 0:S_IDX],
                            num_idxs=TE, num_idxs_reg=TE,
                            elem_size=HID, transpose=True)
                        qTb = sp.tile([128, 2, TE], BF16, tag="qTb", bufs=3)
                        nc.gpsimd.dma_gather(
                            out_ap=qTb[:], in_ap=q_dram[:],
                            idxs_ap=qgi_s[:, t, S_IDX:2 * S_IDX],
                            num_idxs=TE, num_idxs_reg=TE,
                            elem_size=HID, transpose=True)
                        q_lo, q_hi = qTa[:], qTb[:]
                    # scatter one-hots, rebuilt on device from srel:
                    # oT[e, k] = (srel[e] == k) is exactly the sct layout
                    # (edge on partitions); sctT is its PE transpose.
                    oT = sp.tile([128, NSPT, 128], BF16, tag="ohT")
                    for s in range(NSPT):
                        nc.vector.tensor_tensor(
                            out=oT[:, s, :],
                            in0=srel_s[:, t * NSPT + s:t * NSPT + s + 1]
                                .to_broadcast([128, 128]),
                            in1=iota_s[:], op=Alu.is_equal)
                    sct_s = sp.tile([128, NSPT, 128],
                                    F8 if USE_FP8_SCT else BF16, tag="sct")
                    nc.scalar.copy(out=sct_s[:], in_=oT[:])
                    sctT_s = sp.tile([128, NSPT, 128], BF16, tag="sctT")
                    tp8 = psB2.tile([128, NSPT, 128], BF16, tag="tps", bufs=1,
                                    name=f"ohT{t}")
                    for s in range(NSPT):
                        nc.tensor.transpose(out=tp8[:, s, :], in_=oT[:, s, :],
                                            identity=ident[:])
                    nc.vector.tensor_copy(out=sctT_s[:], in_=tp8[:])

                    dT = disT_tiles[t][:]
                    # Q + lat merged on (otherwise idle) DVE: one fewer
                    # PE identity pass per o-chunk.
                    qlT = sp.tile([128, 2, TE], BF16, tag="qlT")
                    nc.vector.tensor_tensor(
                        out=qlT[:], in0=q_lo, in1=q_hi, op=Alu.add)
                    # layer 1 (feature-major); both o-chunks in one psum
                    # tile so silu1 is a single wide ACT call.
                    x1T = sp.tile([128, 2, TE],
                                  F8 if USE_FP8_L2 else BF16, tag="x1T")
                    p1 = ps_mm.tile([128, 2 * TE], F32, tag="mm", name=f"p1_{t}")
                    for oc in range(2):
                        sl1 = slice(oc * TE, (oc + 1) * TE)
                        nc.tensor.matmul(out=p1[:, sl1], lhsT=ident[:],
                                         rhs=qlT[:, oc, :], start=True, stop=False)
                        for s in range(NSPT):
                            w = int(sub2w[t * NSPT + s])
                            nc.tensor.matmul(
                                out=p1[:, oc * TE + s * 128:oc * TE + (s + 1) * 128],
                                lhsT=P_sb[:, w, oc * 128:(oc + 1) * 128],
                                rhs=sctT_s[:, s, :], start=False, stop=False)
                        nc.tensor.matmul(out=p1[:, sl1],
                                         lhsT=Wd_s[:, oc * 128:(oc + 1) * 128],
                                         rhs=dT, start=False, stop=True)
                    nc.scalar.activation(
                        out=x1T[:].rearrange("p c e -> p (c e)"), in_=p1[:],
                        func=Act.Silu)

                    # layer 2 (operand-flipped -> edge-major) + ones column
                    x2_ps = ps_mm.tile([128, NSPT * HID], F32, tag="mm",
                                       name=f"x2ps_{t}")
                    x2 = sp.tile([128, NSPT, HID + 1],
                                 F8 if USE_FP8_SCT else BF16, tag="x2s")
                    for s in range(NSPT):
                        osl = s * HID
                        nc.tensor.matmul(out=x2_ps[:, osl:osl + HID],
                                         lhsT=ones_row[:], rhs=be2_s[:],
                                         start=True, stop=False)
                        if USE_FP8_L2:
                            nc.tensor.matmul(
                                out=x2_ps[:, osl:osl + HID],
                                lhsT=x1T[:, :, s * 128:(s + 1) * 128],
                                rhs=W2_s[:],
                                perf_mode=mybir.MatmulPerfMode.DoubleRow,
                                start=False, stop=True)
                        else:
                            for kc in range(2):
                                nc.tensor.matmul(
                                    out=x2_ps[:, osl:osl + HID],
                                    lhsT=x1T[:, kc, s * 128:(s + 1) * 128],
                                    rhs=W2_s[:, kc, :], start=False,
                                    stop=(kc == 1))
                    nc.scalar.activation(
                        out=x2[:, :, 0:HID],
                        in_=x2_ps[:].rearrange("p (s h) -> p s h", s=NSPT),
                        func=Act.Silu)
                    nc.vector.tensor_copy(out=x2[:, :, HID], in_=ones4[:])

                    # scatter into per-window PSUM accumulators
                    # (DoubleRow pairs two subtiles when both are in the
                    # same window; odd/straddling subtiles go singly)
                    s = 0
                    while s < NSPT:
                        g = t * NSPT + s
                        w = int(sub2w[g])
                        first, last = g == wstart[w], g == wstart[w + 1] - 1
                        pair = (s + 1 < NSPT and not last
                                and int(sub2w[g + 1]) == w)
                        if first:
                            agg_ps[w] = ps_agg.tile([128, HID + 1], F32,
                                                    tag="agg", name=f"agg{w}")
                        if pair and USE_FP8_SCT:
                            last = g + 1 == wstart[w + 1] - 1
                            nc.tensor.matmul(
                                out=agg_ps[w][:],
                                lhsT=sct_s[:, s:s + 2, :],
                                rhs=x2[:, s:s + 2, :],
                                perf_mode=mybir.MatmulPerfMode.DoubleRow,
                                start=first, stop=last,
                                skip_group_check=True)
                            s += 2
                        else:
                            nc.tensor.matmul(out=agg_ps[w][:],
                                             lhsT=sct_s[:, s, :],
                                             rhs=x2[:, s, :], start=first,
                                             stop=last,
                                             skip_group_check=True)
                            s += 1
                        if last:
                            ap = agg_ps.pop(w)
                            cm = sp.tile([128, 1], F32, tag="cnt")
                            nc.vector.tensor_scalar(
                                out=cm[:], in0=ap[:, HID:HID + 1], scalar1=1.0,
                                scalar2=None, op0=Alu.max)
                            nc.vector.reciprocal(out=inv_cnt[:, w:w + 1], in_=cm[:])
                            nc.vector.tensor_tensor(
                                out=agg_bf[:, w, :], in0=ap[:, 0:HID],
                                in1=inv_cnt[:, w:w + 1].to_broadcast([128, HID]),
                                op=Alu.mult)
                            tp = psB2.tile([128, NSPT, 128], BF16, tag="tps",
                                           bufs=1, name=f"aggT{w}")
                            for c2 in range(2):
                                nc.tensor.transpose(
                                    out=tp[:, c2, :],
                                    in_=agg_bf[:, w, c2 * 128:(c2 + 1) * 128],
                                    identity=ident[:])
                                nc.vector.tensor_copy(
                                    out=aggT[:, c2, w * 128:(w + 1) * 128],
                                    in_=tp[:, c2, :])

            # ---------------- phase D: node MLP ----------------
            with tc.tile_pool(name="psD", bufs=1, space="PSUM") as psD:
                h1T = pp.tile([128, 2, NLOC], BF16)
                for oc in range(2):
                    hp = psD.tile([128, NLOC], F32, tag="h1T", bufs=2)
                    for kc in range(4):
                        rhs = nfT_loc_s[:, kc, :] if kc < 2 else aggT[:, kc - 2, :]
                        for nsp in range(0, NLOC, 512):
                            ln = min(512, NLOC - nsp)
                            nc.tensor.matmul(
                                out=hp[:, nsp:nsp + ln],
                                lhsT=Wn1_s[:, kc, oc * 128:(oc + 1) * 128],
                                rhs=rhs[:, nsp:nsp + ln],
                                start=(kc == 0), stop=(kc == 3),
                                skip_group_check=True)
                    nc.scalar.activation(out=h1T[:, oc, :], in_=hp[:],
                                         func=Act.Silu, bias=bn1_s[:, oc:oc + 1])
                outs = pp.tile([128, NW, HID], F32)
                for w in range(NW):
                    op = psD.tile([128, HID], F32, tag="outp", bufs=2)
                    nc.tensor.matmul(out=op[:], lhsT=ones_row[:], rhs=bn2_s[:],
                                     start=True, stop=False)
                    for kc in range(2):
                        nc.tensor.matmul(out=op[:],
                                         lhsT=h1T[:, kc, w * 128:(w + 1) * 128],
                                         rhs=Wn2_s[:, kc, :],
                                         start=False, stop=(kc == 1))
                    o2 = sp.tile([128, HID], F32, tag="o2")
                    nc.scalar.activation(out=o2[:], in_=op[:], func=Act.Silu)
                    nc.vector.tensor_tensor(out=outs[:, w, :], in0=o2[:],
                                            in1=nf_loc_s[:, w, :], op=Alu.add)
                nc.sync.dma_start(out=out_d[:], in_=outs[:])

    return nc


# --------------------------------------------------------------------------
# entry point
# --------------------------------------------------------------------------

def kernel(**inputs):
    from concourse.bass_utils import run_bass_kernel_spmd

    in_maps, meta = _prep(inputs)
    nc = build_program(meta)
    nc.compile()
    res = run_bass_kernel_spmd(nc, in_maps, core_ids=list(range(N_CORES)))
    out = np.zeros((N_NODES, HID), np.float32)
    for c in range(N_CORES):
        o = np.asarray(res.results[c]["out"], np.float32)  # [128, NW, HID]
        o = o.transpose(1, 0, 2).reshape(NLOC, HID)
        out[c * NPC:(c + 1) * NPC] = o[:NPC]
    return out

